# revision 51
# baseline (speedup 1.0000x reference)
"""Causal multi-head attention (B=4, T=2048, H=1024, 16 heads) on 8 trn2 cores.

Sharding: batch(4) x head-group(2).  Core c -> batch b=c//2, heads g=c%2
(8 heads each): zero-communication data/tensor parallelism.  Each core
computes its QKV projection slice, causal+padding-masked attention for its 8
heads, and a row-parallel slice of the output projection; the two partial
outputs per batch row are summed on the host (row-parallel unshard).

Key structural ideas (all bf16 matmuls, f32 PSUM):
  * The kernel is one flat instruction-interleaved pipeline driven by work
    queues: the ScalarE exp stream (the 2nd-largest engine load) starts
    ~15us in and never starves, while PV matmuls of the previous head-pair,
    QK projection tiles for later q-tiles, V-projection chunks and output-
    projection tiles are pumped between S-matmul groups as PE filler.
  * Padding-length truncation: the NEFF is specialized (and cached) on
    m = ceil(max_seq_len/128) computed from the runtime mask; key chunks
    >= m are fully masked for every batch row, so their S/exp/PV/V work is
    skipped entirely (the ones-column denominator trick makes this exact).
  * V carries a ones column per head multiplied by the 0/1 pad mask, so the
    softmax denominator rides along row 64 of the PV psum and padded keys
    drop out of numerator and denominator alike; no -inf masking needed.
  * Causal masking is a post-exp bitwise-AND (uint16 view) with a 0x00/0xFF
    pattern on the DVE; exp of the dd=2,3 diagonal stripes is skipped
    (gpsimd zero-fills) since those columns are mostly below the frontier.
  * The V bias is dropped on device (softmax weights sum to 1 =>  it
    contributes b_v @ w_out to y) and folded into the host-side bias add,
    along with b_out.
"""

import os
import sys

import numpy as np

sys.path.insert(0, "/opt/trn_rl_repo")

B, T, H = 4, 2048, 1024
NH, HD = 16, 64
NCORES = 8
HPC = 8          # heads per core
GD = HPC * HD    # head dims per core = 512
KC = T // 128    # 16 k-chunks
QT_TILES = T // 512  # 4 q-tiles
HC = H // 128    # 8 h-chunks (contraction for projections)


def _build_nc(m_chunks=KC):
    import concourse.bass as bass
    import concourse.tile as tile
    import concourse.mybir as mybir
    from concourse import bacc
    from contextlib import ExitStack
    from collections import deque

    m = max(1, min(KC, int(m_chunks)))

    f32 = mybir.dt.float32
    f32r = mybir.dt.float32r
    bf16 = mybir.dt.bfloat16
    u16 = mybir.dt.uint16
    EXP = mybir.ActivationFunctionType.Exp
    BAND = mybir.AluOpType.bitwise_and

    nc = bacc.Bacc("TRN2", target_bir_lowering=False, debug=False)

    xT_d = nc.dram_tensor("xT", [H, T], bf16, kind="ExternalInput").ap()
    wqk_d = nc.dram_tensor("wqk", [H, 2 * GD], bf16, kind="ExternalInput").ap()
    wqkp_d = nc.dram_tensor("wqkp", [8 * 128, HC * 128], bf16, kind="ExternalInput").ap()
    wvp_d = nc.dram_tensor("wvp", [128, HC * GD], bf16, kind="ExternalInput").ap()
    bqkc_d = nc.dram_tensor("bqkc", [128, 8], f32, kind="ExternalInput").ap()
    wout_d = nc.dram_tensor("wout", [GD, H], f32, kind="ExternalInput").ap()
    padb01_d = nc.dram_tensor("padb01", [128, KC], f32, kind="ExternalInput").ap()
    cmask16_d = nc.dram_tensor("cmask16", [128, 4 * 512], u16, kind="ExternalInput").ap()
    y_d = nc.dram_tensor("y", [T, H], f32, kind="ExternalOutput").ap()

    def r(ap):
        return ap.bitcast(f32r)

    with ExitStack() as ctx:
        tc = ctx.enter_context(tile.TileContext(nc))

        const = ctx.enter_context(tc.tile_pool(name="const", bufs=1))
        padb01_sb = const.tile([128, KC], f32, name="padb01_sb")
        nc.sync.dma_start(padb01_sb, padb01_d)
        bqkc_sb = const.tile([128, 8], f32, name="bqkc_sb")
        nc.sync.dma_start(bqkc_sb, bqkc_d)
        cmask16_sb = const.tile([128, 4 * 512], u16, name="cmask16_sb")

        # Persistent activations
        acts = ctx.enter_context(tc.tile_pool(name="acts", bufs=1))
        qk_sb = [acts.tile([128, T], bf16, name=f"qk{i}") for i in range(8)]
        v_sb = [acts.tile([128, HPC * 65], bf16, name=f"v{c}") for c in range(m)]

        # Operand loading: DMA triggers cost ~600ns each on an engine queue,
        # so transfers are batched (one descriptor per weight group) and
        # spread across the four otherwise-idle queues, ordered so the first
        # projection matmul's working set (x^T halves 0, wqk ct0) lands
        # first.  The first matmul issues at ~8us instead of ~32us.
        xt_pool = ctx.enter_context(tc.tile_pool(name="xt", bufs=1))
        xt = [xt_pool.tile([128, T], bf16, name=f"xt{i}") for i in range(HC)]
        wqk_pool = ctx.enter_context(tc.tile_pool(name="wqkp", bufs=1))
        wts_ct = [wqk_pool.tile([128, HC * 128], bf16, name=f"wct{ct}")
                  for ct in range(8)]
        wv_pool = ctx.enter_context(tc.tile_pool(name="wvp", bufs=1))
        wv_sb = wv_pool.tile([128, HC * GD], bf16, name="wv_sb")
        woutp = ctx.enter_context(tc.tile_pool(name="woutp", bufs=1))
        wout_sb = [woutp.tile([128, H], f32, name=f"wo{hp}") for hp in range(4)]

        for i in range(HC):                       # x^T first quarter (t<512)
            eng = nc.sync if i % 2 == 0 else nc.scalar
            eng.dma_start(xt[i][:, 0:512], xT_d[i * 128:(i + 1) * 128, 0:512])
        for ct in range(8):                       # host-packed: contiguous 2D
            nc.gpsimd.dma_start(
                wts_ct[ct], wqkp_d[ct * 128:(ct + 1) * 128, :])
        nc.gpsimd.dma_start(wv_sb, wvp_d)
        for i in range(HC):                       # x^T rest
            eng = nc.sync if i % 2 == 0 else nc.scalar
            eng.dma_start(xt[i][:, 512:T], xT_d[i * 128:(i + 1) * 128, 512:T])
        for hp in range(4):
            nc.gpsimd.dma_start(r(wout_sb[hp]), r(wout_d[hp * 128:(hp + 1) * 128, :]))
        nc.gpsimd.dma_start(cmask16_sb, cmask16_d)

        ppool = ctx.enter_context(tc.tile_pool(name="pchunks", bufs=14))
        osc_pool = ctx.enter_context(tc.tile_pool(name="osc", bufs=2))
        oden_pool = ctx.enter_context(tc.tile_pool(name="oden", bufs=12))
        dpool = ctx.enter_context(tc.tile_pool(name="dtiles", bufs=2))
        ypool = ctx.enter_context(tc.tile_pool(name="ysb", bufs=3))
        ps_s = ctx.enter_context(tc.tile_pool(name="ps_s", bufs=2, space="PSUM"))
        ps_o = ctx.enter_context(tc.tile_pool(name="ps_o", bufs=2, space="PSUM"))
        ps_w = ctx.enter_context(tc.tile_pool(name="ps_w", bufs=2, space="PSUM"))

        def emit_qk(ct, tt):
            ps = ps_w.tile([128, 512], f32, tag="w", name=f"psqk{ct}_{tt}")
            for hc in range(HC):
                nc.tensor.matmul(
                    ps, wts_ct[ct][:, hc * 128:(hc + 1) * 128],
                    xt[hc][:, tt * 512:(tt + 1) * 512],
                    start=(hc == 0), stop=(hc == HC - 1))
            nc.vector.tensor_scalar_add(
                qk_sb[ct][:, tt * 512:(tt + 1) * 512], ps, bqkc_sb[:, ct:ct + 1])

        def emit_v(ts):
            psv = ps_w.tile([128, 512], f32, tag="w", name=f"psv{ts}")
            for hc in range(HC):
                nc.tensor.matmul(
                    psv, xt[hc][:, ts * 128:(ts + 1) * 128],
                    wv_sb[:, hc * GD:(hc + 1) * GD],
                    start=(hc == 0), stop=(hc == HC - 1))
            pad_c = padb01_sb[:, ts:ts + 1]
            dst = v_sb[ts].rearrange("p (h c) -> p h c", h=HPC)[:, :, 0:64]
            srcv = psv.rearrange("p (h c) -> p h c", h=HPC)
            nc.vector.tensor_scalar_mul(dst, srcv, pad_c)
            ones = v_sb[ts].rearrange("p (h c) -> p h c", h=HPC)[:, :, 64:65]
            nc.vector.memset(ones, 1.0)
            nc.vector.tensor_scalar_mul(ones, ones, pad_c)

        def attn_tail(qt, j, parity, opsum, o_dense):
            """softmax denom -> recip -> broadcast -> scale -> dense repack.
            DVE lanes cannot cross partitions: the denominator (psum
            partition 64) is DVE-copied in place, DMA-shifted to partition
            0, then inverted there."""
            stage = dpool.tile([65, 512], f32, tag="dstage",
                               name=f"st{qt}_{j}_{parity}")
            nc.vector.tensor_copy(stage[64:65, :], opsum[64:65, :])
            dp0 = dpool.tile([1, 512], f32, tag="dp0",
                             name=f"dp0_{qt}_{j}_{parity}")
            nc.sync.dma_start(dp0, stage[64:65, :])
            rp = dpool.tile([1, 512], f32, tag="rp", name=f"rp{qt}_{j}_{parity}")
            nc.vector.reciprocal_approx_fast(rp, dp0)
            rrep = dpool.tile([64, 512], f32, tag="rrep",
                              name=f"rr{qt}_{j}_{parity}")
            nc.gpsimd.partition_broadcast(rrep, rp)
            if parity == 0:
                nc.vector.tensor_mul(r(o_dense[0:64, :]), rrep, opsum[0:64, :])
            else:
                o_sc = osc_pool.tile([64, 512], f32, tag="osc",
                                     name=f"osc{qt}_{j}")
                nc.vector.tensor_mul(o_sc, rrep, opsum[0:64, :])
                nc.sync.dma_start(r(o_dense[64:128, :]), r(o_sc))

        def emit_y_tile(qt, jj, ts, oden):
            """one output-projection tile (host adds the bias)"""
            q0 = qt * 512
            ypsum = ps_w.tile([128, 512], f32, tag="w", name=f"y{qt}_{jj}_{ts}")
            for hp in range(4):
                nc.tensor.matmul(
                    ypsum,
                    r(oden[hp][:, ts * 128:(ts + 1) * 128]),
                    r(wout_sb[hp][:, jj * 512:(jj + 1) * 512]),
                    start=(hp == 0), stop=(hp == 3))
            ysb = ypool.tile([128, 512], f32, tag="ysb", name=f"ys{qt}_{jj}_{ts}")
            if qt == 2:
                # qt2's y tiles drain while qt3's exp stream is the binding
                # engine -- keep ScalarE clear there, DVE has slack
                nc.vector.tensor_copy(ysb, ypsum)
            else:
                nc.scalar.copy(ysb, ypsum)
            nc.sync.dma_start(
                y_d[q0 + ts * 128:q0 + (ts + 1) * 128, jj * 512:(jj + 1) * 512],
                ysb)

        # ---------------- Phase 1-lite: QK tt=0 + V chunks 0..3 ----------
        for ct in range(8):
            emit_qk(ct, 0)
        for ts in range(min(4, m)):
            emit_v(ts)

        # ---------------- Attention pipeline ----------------
        pv_q = deque()      # PV matmuls + softmax tails of the previous pair
        braid_q = deque()   # projection filler (QK tt>=1, V chunks)
        y_q = deque()       # output-projection tiles of the previous q-tile

        def pump(n_pv, n_braid):
            for _ in range(n_pv):
                if pv_q:
                    pv_q.popleft()()
            for _ in range(n_braid):
                if braid_q:
                    braid_q.popleft()()
                elif y_q:
                    y_q.popleft()()
                elif pv_q:
                    pv_q.popleft()()

        def make_pv_items(qt, j, nk_eff, pts, o_dense):
            items = []
            for p in range(2):
                h = 2 * j + p
                holder = {}
                for c in range(nk_eff):
                    def mm(c=c, p=p, h=h, holder=holder, qt=qt, j=j,
                           nk_eff=nk_eff, pts=pts):
                        if c == 0:
                            holder["ps"] = ps_o.tile(
                                [65, 512], f32, tag="o", name=f"o{qt}_{j}_{p}")
                        nc.tensor.matmul(
                            holder["ps"],
                            v_sb[c][:, h * 65:(h + 1) * 65],
                            pts[p][c // 2][:, (c % 2) * 512:(c % 2) * 512 + 512],
                            start=(c == 0), stop=(c == nk_eff - 1))
                    items.append(mm)

                def tail(p=p, holder=holder):
                    attn_tail(qt, j, p, holder["ps"], o_dense)
                items.append(tail)
            return items

        oden_by_qt = {}
        for qt in range(QT_TILES):
            q0 = qt * 512
            nk = 4 * (qt + 1)
            nk_eff = min(nk, m)
            ngroups = (nk_eff + 1) // 2

            # refill the braid queues for this q-tile
            if qt >= 2:
                while y_q:           # oden slots of qt-2 are reused below
                    y_q.popleft()()
            if qt == 0:
                for ct in range(8):
                    braid_q.append(lambda ct=ct: emit_qk(ct, 1))
                for ts in range(4, min(8, m)):
                    braid_q.append(lambda ts=ts: emit_v(ts))
            else:
                for ts in range(4 * qt + 4, min(4 * qt + 8, m)):
                    braid_q.append(lambda ts=ts: emit_v(ts))
                if qt < 3:
                    for ct in range(8):
                        braid_q.append(
                            lambda ct=ct, tt=qt + 1: emit_qk(ct, tt))

            oden = []
            oden_by_qt[qt] = oden
            n_braid = 2 if qt == 0 else 1
            for j in range(4):   # head pair (2j, 2j+1)
                if j == 1 and qt >= 1:
                    # the previous q-tile's PV+tails have fully drained from
                    # pv_q by now (FIFO ahead of this q-tile's pair 0), so
                    # its output-projection tiles may enter the pump.
                    oden_prev = oden_by_qt[qt - 1]
                    for jj in range(2):
                        for ts in range(4):
                            y_q.append(
                                lambda qtp=qt - 1, jj=jj, ts=ts,
                                oden=oden_prev:
                                emit_y_tile(qtp, jj, ts, oden))
                o_dense = oden_pool.tile([128, 512], f32, tag="od",
                                         name=f"od{qt}_{j}")
                oden.append(o_dense)
                hq = [qk_sb[j][p * 64:(p + 1) * 64, q0:q0 + 512]
                      for p in range(2)]
                sps = [None, None]
                pts = [[], []]
                for cc in range(ngroups):
                    c1_exists = 2 * cc + 1 < nk_eff
                    for p in range(2):
                        sps[p] = ps_s.tile([128, 1024], f32, tag="s",
                                           name=f"s{qt}_{j}_{cc}_{p}")
                    for ci in range(2 if c1_exists else 1):
                        c = 2 * cc + ci
                        # columns q < 128*dd of a diagonal chunk are entirely
                        # below the causal frontier -- skip them in the S
                        # matmul (and in exp below; gpsimd zero-fills P).
                        dlo = 128 * (c - 4 * qt) if c >= 4 * qt else 0
                        for p in range(2):
                            hk = qk_sb[4 + j][p * 64:(p + 1) * 64,
                                              c * 128:(c + 1) * 128]
                            nc.tensor.matmul(
                                sps[p][:, ci * 512 + dlo:(ci + 1) * 512],
                                hk, hq[p][:, dlo:512], start=True, stop=True)
                    tri = (cc == 2 * qt + 1) and c1_exists
                    dia = (cc == 2 * qt) and qt > 0 or (qt == 0 and cc == 0)
                    for p in range(2):
                        pt = ppool.tile([128, 1024], bf16, tag="p",
                                        name=f"p{qt}_{j}_{cc}_{p}")
                        if tri:
                            # diagonal chunks dd=2,3: skip exp of the
                            # always-invalid columns; gpsimd zero-fills
                            # them so PV reads clean zeros.
                            nc.scalar.activation(pt[:, 256:512],
                                                 sps[p][:, 256:512], EXP,
                                                 bias=0.0, scale=1.0)
                            nc.scalar.activation(pt[:, 896:1024],
                                                 sps[p][:, 896:1024], EXP,
                                                 bias=0.0, scale=1.0)
                            nc.gpsimd.memset(pt[:, 0:256], 0.0)
                            nc.gpsimd.memset(pt[:, 512:896], 0.0)
                        elif dia and c1_exists:
                            # dd=0 chunk full, dd=1 chunk skips cols < 128
                            nc.scalar.activation(pt[:, 0:512],
                                                 sps[p][:, 0:512], EXP,
                                                 bias=0.0, scale=1.0)
                            nc.scalar.activation(pt[:, 640:1024],
                                                 sps[p][:, 640:1024], EXP,
                                                 bias=0.0, scale=1.0)
                            nc.gpsimd.memset(pt[:, 512:640], 0.0)
                        elif c1_exists:
                            nc.scalar.activation(pt, sps[p], EXP,
                                                 bias=0.0, scale=1.0)
                        else:
                            nc.scalar.activation(pt[:, 0:512],
                                                 sps[p][:, 0:512], EXP,
                                                 bias=0.0, scale=1.0)
                        for ci in range(2 if c1_exists else 1):
                            c = 2 * cc + ci
                            if c >= 4 * qt:
                                # causal mask: bitwise-AND (uint16 view)
                                # with a 0x0000/0xFFFF pattern -- exact,
                                # and 2-byte dtype keeps the DVE in 2x mode.
                                dd = c - 4 * qt
                                lo = 128 * dd
                                sl = pt[:, ci * 512 + lo:
                                        (ci + 1) * 512].bitcast(u16)
                                cm = cmask16_sb[:, dd * 512 + lo:
                                                (dd + 1) * 512]
                                nc.vector.tensor_tensor(sl, sl, cm, BAND)
                        pts[p].append(pt)
                    pump(5, n_braid)
                pv_q.extend(make_pv_items(qt, j, nk_eff, pts, o_dense))

        # drain: last pair's PV + tails, leftover filler, last y tiles
        while pv_q:
            pv_q.popleft()()
        while braid_q:
            braid_q.popleft()()
        while y_q:
            y_q.popleft()()
        for jj in range(2):
            for ts in range(4):
                emit_y_tile(3, jj, ts, oden_by_qt[3])

    nc.compile()
    return nc


_NC_CACHE = {}
_NC_LAST = None


def _get_nc(m_chunks=None):
    global _NC_LAST
    if m_chunks is None:
        if _NC_LAST is not None:
            return _NC_LAST
        m_chunks = KC
    if m_chunks not in _NC_CACHE:
        _NC_CACHE[m_chunks] = _build_nc(m_chunks)
    _NC_LAST = _NC_CACHE[m_chunks]
    return _NC_LAST


def make_core_inputs(input, mask, w_qkv, b_qkv, w_out, b_out, core):
    """Host-side sharding/layout prep for one core."""
    b, g = core // 2, core % 2
    scale = 1.0 / np.sqrt(HD)

    import ml_dtypes
    xT = np.ascontiguousarray(input[b].T).astype(ml_dtypes.bfloat16)  # [H, T]

    qcols = slice(g * GD, (g + 1) * GD)
    kcols = slice(H + g * GD, H + (g + 1) * GD)
    vcols = slice(2 * H + g * GD, 2 * H + (g + 1) * GD)
    wq = w_qkv[:, qcols] * scale
    wk = w_qkv[:, kcols]
    wqk = np.ascontiguousarray(np.concatenate([wq, wk], axis=1)).astype(ml_dtypes.bfloat16)
    # packed [ct*128 + p, hc*128 + c] = wqk[hc*128 + p, ct*128 + c]
    wqkp = np.ascontiguousarray(
        np.asarray(wqk).reshape(HC, 128, 8, 128).transpose(2, 1, 0, 3)
        .reshape(8 * 128, HC * 128))
    bqk = np.concatenate([b_qkv[qcols] * scale, b_qkv[kcols]]).astype(np.float32)
    bqkc = np.ascontiguousarray(bqk.reshape(8, 128).T)               # [128, 8]
    wv = np.asarray(w_qkv[:, vcols]).astype(ml_dtypes.bfloat16)
    # packed [p, hc*GD + c] = wv[hc*128 + p, c]
    wvp = np.ascontiguousarray(
        np.asarray(wv).reshape(HC, 128, GD).transpose(1, 0, 2).reshape(128, HC * GD))

    wout = np.ascontiguousarray(w_out[g * GD:(g + 1) * GD, :]).astype(np.float32)

    padb01 = mask[b].astype(np.float32)                                # [T]
    padb01 = np.ascontiguousarray(padb01.reshape(KC, 128).T)           # [128, KC]

    # 4 causal diagonal mask patterns as 0x0000/0xFFFF uint16 (bf16-element
    # AND masks): valid iff col >= row + 128*dd
    rr = np.arange(128)[:, None]
    cc = np.arange(512)[None, :]
    cm = np.empty((128, 4 * 512), dtype=np.uint16)
    for dd in range(4):
        cm[:, dd * 512:(dd + 1) * 512] = np.where(
            cc >= rr + 128 * dd, np.uint16(0xFFFF), np.uint16(0))
    cmask16 = cm

    return {
        "xT": xT, "wqk": wqk, "wqkp": wqkp, "wvp": wvp, "bqkc": bqkc,
        "wout": wout, "padb01": padb01, "cmask16": cmask16,
    }


def _host_bias(w_qkv, b_qkv, w_out, b_out):
    """b_out plus the folded V-projection bias: softmax weights sum to 1, so
    the V bias contributes b_v @ w_out to every output row."""
    bv = b_qkv[2 * H:3 * H].astype(np.float64)
    return (b_out.astype(np.float64) + bv @ w_out.astype(np.float64)).astype(np.float32)


def finalize_partial(y, core, inputs):
    """Host-side finalization of one core's raw y output (for sim_test)."""
    g = core % 2
    bv_g = inputs["b_qkv"][2 * H + g * GD:2 * H + (g + 1) * GD].astype(np.float64)
    wout_g = inputs["w_out"][g * GD:(g + 1) * GD].astype(np.float64)
    out = y.astype(np.float64) + bv_g @ wout_g
    if g == 0:
        out = out + inputs["b_out"].astype(np.float64)
    return out.astype(np.float32)


def _mask_chunks(mask):
    """ceil(max valid length / 128) over the batch -- the NEFF specializer."""
    lens = np.asarray(mask).astype(np.int64).sum(axis=1)
    return int(max(1, min(KC, int(np.ceil(lens.max() / 128.0)))))


def kernel(input, mask, w_qkv, b_qkv, w_out, b_out):
    from concourse.bass_utils import run_bass_kernel_spmd

    input = np.asarray(input)
    mask = np.asarray(mask)
    w_qkv = np.asarray(w_qkv)
    b_qkv = np.asarray(b_qkv)
    w_out = np.asarray(w_out)
    b_out = np.asarray(b_out)
    nc = _get_nc(_mask_chunks(mask))
    in_maps = [
        make_core_inputs(input, mask, w_qkv, b_qkv, w_out, b_out, c)
        for c in range(NCORES)
    ]
    res = run_bass_kernel_spmd(nc, in_maps, list(range(NCORES)))
    bias = _host_bias(w_qkv, b_qkv, w_out, b_out)
    parts = [res.results[c]["y"] for c in range(NCORES)]
    out = np.stack([parts[2 * b] + parts[2 * b + 1] + bias for b in range(B)])
    return out.astype(np.float32)


if __name__ == "__main__":
    nc = _build_nc(13)
    print("build ok")


# revision 54
# speedup vs baseline: 1.0196x; 1.0196x over previous
"""Causal multi-head attention (B=4, T=2048, H=1024, 16 heads) on 8 trn2 cores.

Sharding: batch(4) x head-group(2).  Core c -> batch b=c//2, heads g=c%2
(8 heads each): zero-communication data/tensor parallelism.  Each core
computes its QKV projection slice, causal+padding-masked attention for its 8
heads, and a row-parallel slice of the output projection; the two partial
outputs per batch row are summed on the host (row-parallel unshard).

Key structural ideas (all bf16 matmuls, f32 PSUM):
  * The kernel is one flat instruction-interleaved pipeline driven by work
    queues: the ScalarE exp stream (the 2nd-largest engine load) starts
    ~15us in and never starves, while PV matmuls of the previous head-pair,
    QK projection tiles for later q-tiles, V-projection chunks and output-
    projection tiles are pumped between S-matmul groups as PE filler.
  * Padding-length truncation: the NEFF is specialized (and cached) on
    m = ceil(max_seq_len/128) computed from the runtime mask; key chunks
    >= m are fully masked for every batch row, so their S/exp/PV/V work is
    skipped entirely (the ones-column denominator trick makes this exact).
  * V carries a ones column per head multiplied by the 0/1 pad mask, so the
    softmax denominator rides along row 64 of the PV psum and padded keys
    drop out of numerator and denominator alike; no -inf masking needed.
  * Causal masking is a post-exp bitwise-AND (uint16 view) with a 0x00/0xFF
    pattern on the DVE; exp of the dd=2,3 diagonal stripes is skipped
    (gpsimd zero-fills) since those columns are mostly below the frontier.
  * The V bias is dropped on device (softmax weights sum to 1 =>  it
    contributes b_v @ w_out to y) and folded into the host-side bias add,
    along with b_out.
"""

import os
import sys

import numpy as np

sys.path.insert(0, "/opt/trn_rl_repo")

B, T, H = 4, 2048, 1024
NH, HD = 16, 64
NCORES = 8
HPC = 8          # heads per core
GD = HPC * HD    # head dims per core = 512
KC = T // 128    # 16 k-chunks
QT_TILES = T // 512  # 4 q-tiles
HC = H // 128    # 8 h-chunks (contraction for projections)


def _build_nc(m_chunks=KC):
    import concourse.bass as bass
    import concourse.tile as tile
    import concourse.mybir as mybir
    from concourse import bacc
    from contextlib import ExitStack
    from collections import deque

    m = max(1, min(KC, int(m_chunks)))

    f32 = mybir.dt.float32
    f32r = mybir.dt.float32r
    bf16 = mybir.dt.bfloat16
    u16 = mybir.dt.uint16
    EXP = mybir.ActivationFunctionType.Exp
    BAND = mybir.AluOpType.bitwise_and

    nc = bacc.Bacc("TRN2", target_bir_lowering=False, debug=False)

    xT_d = nc.dram_tensor("xT", [H, T], bf16, kind="ExternalInput").ap()
    wqk_d = nc.dram_tensor("wqk", [H, 2 * GD], bf16, kind="ExternalInput").ap()
    wqkp_d = nc.dram_tensor("wqkp", [8 * 128, HC * 128], bf16, kind="ExternalInput").ap()
    wvp_d = nc.dram_tensor("wvp", [128, HC * GD], bf16, kind="ExternalInput").ap()
    bqkc_d = nc.dram_tensor("bqkc", [128, 8], f32, kind="ExternalInput").ap()
    wout_d = nc.dram_tensor("wout", [GD, H], f32, kind="ExternalInput").ap()
    padb01_d = nc.dram_tensor("padb01", [128, KC], f32, kind="ExternalInput").ap()
    cmask16_d = nc.dram_tensor("cmask16", [128, 4 * 512], u16, kind="ExternalInput").ap()
    y_d = nc.dram_tensor("y", [T, H], f32, kind="ExternalOutput").ap()

    def r(ap):
        return ap.bitcast(f32r)

    with ExitStack() as ctx:
        tc = ctx.enter_context(tile.TileContext(nc))

        const = ctx.enter_context(tc.tile_pool(name="const", bufs=1))
        padb01_sb = const.tile([128, KC], f32, name="padb01_sb")
        nc.sync.dma_start(padb01_sb, padb01_d)
        bqkc_sb = const.tile([128, 8], f32, name="bqkc_sb")
        nc.sync.dma_start(bqkc_sb, bqkc_d)
        cmask16_sb = const.tile([128, 4 * 512], u16, name="cmask16_sb")

        # Persistent activations
        acts = ctx.enter_context(tc.tile_pool(name="acts", bufs=1))
        qk_sb = [acts.tile([128, T], bf16, name=f"qk{i}") for i in range(8)]
        v_sb = [acts.tile([128, HPC * 65], bf16, name=f"v{c}") for c in range(m)]

        # Operand loading: DMA triggers cost ~600ns each on an engine queue,
        # so transfers are batched (one descriptor per weight group) and
        # spread across the four otherwise-idle queues, ordered so the first
        # projection matmul's working set (x^T halves 0, wqk ct0) lands
        # first.  The first matmul issues at ~8us instead of ~32us.
        xt_pool = ctx.enter_context(tc.tile_pool(name="xt", bufs=1))
        xt = [xt_pool.tile([128, T], bf16, name=f"xt{i}") for i in range(HC)]
        wqk_pool = ctx.enter_context(tc.tile_pool(name="wqkp", bufs=1))
        wts_ct = [wqk_pool.tile([128, HC * 128], bf16, name=f"wct{ct}")
                  for ct in range(8)]
        wv_pool = ctx.enter_context(tc.tile_pool(name="wvp", bufs=1))
        wv_sb = wv_pool.tile([128, HC * GD], bf16, name="wv_sb")
        woutp = ctx.enter_context(tc.tile_pool(name="woutp", bufs=1))
        wout_sb = [woutp.tile([128, H], f32, name=f"wo{hp}") for hp in range(4)]

        for i in range(HC):                       # x^T first quarter (t<512)
            eng = nc.sync if i % 2 == 0 else nc.scalar
            eng.dma_start(xt[i][:, 0:512], xT_d[i * 128:(i + 1) * 128, 0:512])
        for ct in range(8):                       # host-packed: contiguous 2D
            nc.gpsimd.dma_start(
                wts_ct[ct], wqkp_d[ct * 128:(ct + 1) * 128, :])
        nc.gpsimd.dma_start(wv_sb, wvp_d)
        for i in range(HC):                       # x^T rest
            eng = nc.sync if i % 2 == 0 else nc.scalar
            eng.dma_start(xt[i][:, 512:T], xT_d[i * 128:(i + 1) * 128, 512:T])
        for hp in range(4):
            nc.gpsimd.dma_start(r(wout_sb[hp]), r(wout_d[hp * 128:(hp + 1) * 128, :]))
        nc.gpsimd.dma_start(cmask16_sb, cmask16_d)

        ppool = ctx.enter_context(tc.tile_pool(name="pchunks", bufs=14))
        osc_pool = ctx.enter_context(tc.tile_pool(name="osc", bufs=2))
        oden_pool = ctx.enter_context(tc.tile_pool(name="oden", bufs=12))
        dpool = ctx.enter_context(tc.tile_pool(name="dtiles", bufs=2))
        ypool = ctx.enter_context(tc.tile_pool(name="ysb", bufs=3))
        ps_s = ctx.enter_context(tc.tile_pool(name="ps_s", bufs=2, space="PSUM"))
        ps_o = ctx.enter_context(tc.tile_pool(name="ps_o", bufs=2, space="PSUM"))
        ps_w = ctx.enter_context(tc.tile_pool(name="ps_w", bufs=2, space="PSUM"))

        def emit_qk(ct, tt):
            ps = ps_w.tile([128, 512], f32, tag="w", name=f"psqk{ct}_{tt}")
            for hc in range(HC):
                nc.tensor.matmul(
                    ps, wts_ct[ct][:, hc * 128:(hc + 1) * 128],
                    xt[hc][:, tt * 512:(tt + 1) * 512],
                    start=(hc == 0), stop=(hc == HC - 1))
            nc.vector.tensor_scalar_add(
                qk_sb[ct][:, tt * 512:(tt + 1) * 512], ps, bqkc_sb[:, ct:ct + 1])

        def emit_v(ts):
            psv = ps_w.tile([128, 512], f32, tag="w", name=f"psv{ts}")
            for hc in range(HC):
                nc.tensor.matmul(
                    psv, xt[hc][:, ts * 128:(ts + 1) * 128],
                    wv_sb[:, hc * GD:(hc + 1) * GD],
                    start=(hc == 0), stop=(hc == HC - 1))
            pad_c = padb01_sb[:, ts:ts + 1]
            dst = v_sb[ts].rearrange("p (h c) -> p h c", h=HPC)[:, :, 0:64]
            srcv = psv.rearrange("p (h c) -> p h c", h=HPC)
            nc.vector.tensor_scalar_mul(dst, srcv, pad_c)
            ones = v_sb[ts].rearrange("p (h c) -> p h c", h=HPC)[:, :, 64:65]
            nc.vector.memset(ones, 1.0)
            nc.vector.tensor_scalar_mul(ones, ones, pad_c)

        def attn_tail(qt, j, parity, opsum, o_dense):
            """softmax denom -> recip -> broadcast -> scale -> dense repack.
            DVE lanes cannot cross partitions: the denominator (psum
            partition 64) is DVE-copied in place, DMA-shifted to partition
            0, then inverted there."""
            stage = dpool.tile([65, 512], f32, tag="dstage",
                               name=f"st{qt}_{j}_{parity}")
            nc.vector.tensor_copy(stage[64:65, :], opsum[64:65, :])
            dp0 = dpool.tile([1, 512], f32, tag="dp0",
                             name=f"dp0_{qt}_{j}_{parity}")
            nc.sync.dma_start(dp0, stage[64:65, :])
            rp = dpool.tile([1, 512], f32, tag="rp", name=f"rp{qt}_{j}_{parity}")
            nc.vector.reciprocal_approx_fast(rp, dp0)
            rrep = dpool.tile([64, 512], f32, tag="rrep",
                              name=f"rr{qt}_{j}_{parity}")
            nc.gpsimd.partition_broadcast(rrep, rp)
            if parity == 0:
                nc.vector.tensor_mul(r(o_dense[0:64, :]), rrep, opsum[0:64, :])
            else:
                o_sc = osc_pool.tile([64, 512], f32, tag="osc",
                                     name=f"osc{qt}_{j}")
                nc.vector.tensor_mul(o_sc, rrep, opsum[0:64, :])
                nc.sync.dma_start(r(o_dense[64:128, :]), r(o_sc))

        def emit_y_tile(qt, jj, ts, oden):
            """one output-projection tile (host adds the bias)"""
            q0 = qt * 512
            ypsum = ps_w.tile([128, 512], f32, tag="w", name=f"y{qt}_{jj}_{ts}")
            for hp in range(4):
                nc.tensor.matmul(
                    ypsum,
                    r(oden[hp][:, ts * 128:(ts + 1) * 128]),
                    r(wout_sb[hp][:, jj * 512:(jj + 1) * 512]),
                    start=(hp == 0), stop=(hp == 3))
            ysb = ypool.tile([128, 512], f32, tag="ysb", name=f"ys{qt}_{jj}_{ts}")
            nc.scalar.copy(ysb, ypsum)
            nc.sync.dma_start(
                y_d[q0 + ts * 128:q0 + (ts + 1) * 128, jj * 512:(jj + 1) * 512],
                ysb)

        # ---------------- Phase 1-lite: QK tt=0 + V chunks 0..3 ----------
        for ct in range(8):
            emit_qk(ct, 0)
        for ts in range(min(4, m)):
            emit_v(ts)

        # ---------------- Attention pipeline ----------------
        pv_q = deque()      # PV matmuls + softmax tails of the previous pair
        braid_q = deque()   # projection filler (QK tt>=1, V chunks)
        y_q = deque()       # output-projection tiles of the previous q-tile

        def pump(n_pv, n_braid):
            for _ in range(n_pv):
                if pv_q:
                    pv_q.popleft()()
            for _ in range(n_braid):
                if braid_q:
                    braid_q.popleft()()
                elif y_q:
                    y_q.popleft()()
                elif pv_q:
                    pv_q.popleft()()

        def make_pv_items(qt, j, nk_eff, pts, o_dense):
            items = []
            for p in range(2):
                h = 2 * j + p
                holder = {}
                for c in range(nk_eff):
                    def mm(c=c, p=p, h=h, holder=holder, qt=qt, j=j,
                           nk_eff=nk_eff, pts=pts):
                        if c == 0:
                            holder["ps"] = ps_o.tile(
                                [65, 512], f32, tag="o", name=f"o{qt}_{j}_{p}")
                        nc.tensor.matmul(
                            holder["ps"],
                            v_sb[c][:, h * 65:(h + 1) * 65],
                            pts[p][c // 2][:, (c % 2) * 512:(c % 2) * 512 + 512],
                            start=(c == 0), stop=(c == nk_eff - 1))
                    items.append(mm)

                def tail(p=p, holder=holder):
                    attn_tail(qt, j, p, holder["ps"], o_dense)
                items.append(tail)
            return items

        oden_by_qt = {}
        for qt in range(QT_TILES):
            q0 = qt * 512
            nk = 4 * (qt + 1)
            nk_eff = min(nk, m)
            ngroups = (nk_eff + 1) // 2

            # refill the braid queues for this q-tile
            if qt >= 2:
                while y_q:           # oden slots of qt-2 are reused below
                    y_q.popleft()()
            if qt == 0:
                for ct in range(8):
                    braid_q.append(lambda ct=ct: emit_qk(ct, 1))
                for ts in range(4, min(8, m)):
                    braid_q.append(lambda ts=ts: emit_v(ts))
            else:
                for ts in range(4 * qt + 4, min(4 * qt + 8, m)):
                    braid_q.append(lambda ts=ts: emit_v(ts))
                if qt < 3:
                    for ct in range(8):
                        braid_q.append(
                            lambda ct=ct, tt=qt + 1: emit_qk(ct, tt))

            oden = []
            oden_by_qt[qt] = oden
            n_braid = 2 if qt == 0 else 1
            for j in range(4):   # head pair (2j, 2j+1)
                if j == 1 and qt >= 1:
                    # the previous q-tile's PV+tails have fully drained from
                    # pv_q by now (FIFO ahead of this q-tile's pair 0), so
                    # its output-projection tiles may enter the pump.
                    oden_prev = oden_by_qt[qt - 1]
                    for jj in range(2):
                        for ts in range(4):
                            y_q.append(
                                lambda qtp=qt - 1, jj=jj, ts=ts,
                                oden=oden_prev:
                                emit_y_tile(qtp, jj, ts, oden))
                o_dense = oden_pool.tile([128, 512], f32, tag="od",
                                         name=f"od{qt}_{j}")
                oden.append(o_dense)
                hq = [qk_sb[j][p * 64:(p + 1) * 64, q0:q0 + 512]
                      for p in range(2)]
                sps = [None, None]
                pts = [[], []]
                for cc in range(ngroups):
                    c1_exists = 2 * cc + 1 < nk_eff
                    for p in range(2):
                        sps[p] = ps_s.tile([128, 1024], f32, tag="s",
                                           name=f"s{qt}_{j}_{cc}_{p}")
                    for ci in range(2 if c1_exists else 1):
                        c = 2 * cc + ci
                        # columns q < 128*dd of a diagonal chunk are entirely
                        # below the causal frontier -- skip them in the S
                        # matmul (and in exp below; gpsimd zero-fills P).
                        dlo = 128 * (c - 4 * qt) if c >= 4 * qt else 0
                        for p in range(2):
                            hk = qk_sb[4 + j][p * 64:(p + 1) * 64,
                                              c * 128:(c + 1) * 128]
                            nc.tensor.matmul(
                                sps[p][:, ci * 512 + dlo:(ci + 1) * 512],
                                hk, hq[p][:, dlo:512], start=True, stop=True)
                    tri = (cc == 2 * qt + 1) and c1_exists
                    dia = (cc == 2 * qt) and qt > 0 or (qt == 0 and cc == 0)
                    for p in range(2):
                        pt = ppool.tile([128, 1024], bf16, tag="p",
                                        name=f"p{qt}_{j}_{cc}_{p}")
                        if tri:
                            # diagonal chunks dd=2,3: skip exp of the
                            # always-invalid columns; gpsimd zero-fills
                            # them so PV reads clean zeros.
                            nc.scalar.activation(pt[:, 256:512],
                                                 sps[p][:, 256:512], EXP,
                                                 bias=0.0, scale=1.0)
                            nc.scalar.activation(pt[:, 896:1024],
                                                 sps[p][:, 896:1024], EXP,
                                                 bias=0.0, scale=1.0)
                            nc.gpsimd.memset(pt[:, 0:256], 0.0)
                            nc.gpsimd.memset(pt[:, 512:896], 0.0)
                        elif dia and c1_exists:
                            # dd=0 chunk full, dd=1 chunk skips cols < 128
                            nc.scalar.activation(pt[:, 0:512],
                                                 sps[p][:, 0:512], EXP,
                                                 bias=0.0, scale=1.0)
                            nc.scalar.activation(pt[:, 640:1024],
                                                 sps[p][:, 640:1024], EXP,
                                                 bias=0.0, scale=1.0)
                            nc.gpsimd.memset(pt[:, 512:640], 0.0)
                        elif c1_exists:
                            nc.scalar.activation(pt, sps[p], EXP,
                                                 bias=0.0, scale=1.0)
                        else:
                            nc.scalar.activation(pt[:, 0:512],
                                                 sps[p][:, 0:512], EXP,
                                                 bias=0.0, scale=1.0)
                        for ci in range(2 if c1_exists else 1):
                            c = 2 * cc + ci
                            if c >= 4 * qt:
                                # causal mask: bitwise-AND (uint16 view)
                                # with a 0x0000/0xFFFF pattern -- exact,
                                # and 2-byte dtype keeps the DVE in 2x mode.
                                dd = c - 4 * qt
                                lo = 128 * dd
                                sl = pt[:, ci * 512 + lo:
                                        (ci + 1) * 512].bitcast(u16)
                                cm = cmask16_sb[:, dd * 512 + lo:
                                                (dd + 1) * 512]
                                nc.vector.tensor_tensor(sl, sl, cm, BAND)
                        pts[p].append(pt)
                    pump(5, n_braid)
                pv_q.extend(make_pv_items(qt, j, nk_eff, pts, o_dense))

        # drain: last pair's PV + tails, leftover filler, last y tiles
        while pv_q:
            pv_q.popleft()()
        while braid_q:
            braid_q.popleft()()
        while y_q:
            y_q.popleft()()
        for jj in range(2):
            for ts in range(4):
                emit_y_tile(3, jj, ts, oden_by_qt[3])

    nc.compile()
    return nc


_NC_CACHE = {}
_NC_LAST = None


def _get_nc(m_chunks=None):
    global _NC_LAST
    if m_chunks is None:
        if _NC_LAST is not None:
            return _NC_LAST
        m_chunks = KC
    if m_chunks not in _NC_CACHE:
        _NC_CACHE[m_chunks] = _build_nc(m_chunks)
    _NC_LAST = _NC_CACHE[m_chunks]
    return _NC_LAST


def make_core_inputs(input, mask, w_qkv, b_qkv, w_out, b_out, core):
    """Host-side sharding/layout prep for one core."""
    b, g = core // 2, core % 2
    scale = 1.0 / np.sqrt(HD)

    import ml_dtypes
    xT = np.ascontiguousarray(input[b].T).astype(ml_dtypes.bfloat16)  # [H, T]

    qcols = slice(g * GD, (g + 1) * GD)
    kcols = slice(H + g * GD, H + (g + 1) * GD)
    vcols = slice(2 * H + g * GD, 2 * H + (g + 1) * GD)
    wq = w_qkv[:, qcols] * scale
    wk = w_qkv[:, kcols]
    wqk = np.ascontiguousarray(np.concatenate([wq, wk], axis=1)).astype(ml_dtypes.bfloat16)
    # packed [ct*128 + p, hc*128 + c] = wqk[hc*128 + p, ct*128 + c]
    wqkp = np.ascontiguousarray(
        np.asarray(wqk).reshape(HC, 128, 8, 128).transpose(2, 1, 0, 3)
        .reshape(8 * 128, HC * 128))
    bqk = np.concatenate([b_qkv[qcols] * scale, b_qkv[kcols]]).astype(np.float32)
    bqkc = np.ascontiguousarray(bqk.reshape(8, 128).T)               # [128, 8]
    wv = np.asarray(w_qkv[:, vcols]).astype(ml_dtypes.bfloat16)
    # packed [p, hc*GD + c] = wv[hc*128 + p, c]
    wvp = np.ascontiguousarray(
        np.asarray(wv).reshape(HC, 128, GD).transpose(1, 0, 2).reshape(128, HC * GD))

    wout = np.ascontiguousarray(w_out[g * GD:(g + 1) * GD, :]).astype(np.float32)

    padb01 = mask[b].astype(np.float32)                                # [T]
    padb01 = np.ascontiguousarray(padb01.reshape(KC, 128).T)           # [128, KC]

    # 4 causal diagonal mask patterns as 0x0000/0xFFFF uint16 (bf16-element
    # AND masks): valid iff col >= row + 128*dd
    rr = np.arange(128)[:, None]
    cc = np.arange(512)[None, :]
    cm = np.empty((128, 4 * 512), dtype=np.uint16)
    for dd in range(4):
        cm[:, dd * 512:(dd + 1) * 512] = np.where(
            cc >= rr + 128 * dd, np.uint16(0xFFFF), np.uint16(0))
    cmask16 = cm

    return {
        "xT": xT, "wqk": wqk, "wqkp": wqkp, "wvp": wvp, "bqkc": bqkc,
        "wout": wout, "padb01": padb01, "cmask16": cmask16,
    }


def _host_bias(w_qkv, b_qkv, w_out, b_out):
    """b_out plus the folded V-projection bias: softmax weights sum to 1, so
    the V bias contributes b_v @ w_out to every output row."""
    bv = b_qkv[2 * H:3 * H].astype(np.float64)
    return (b_out.astype(np.float64) + bv @ w_out.astype(np.float64)).astype(np.float32)


def finalize_partial(y, core, inputs):
    """Host-side finalization of one core's raw y output (for sim_test)."""
    g = core % 2
    bv_g = inputs["b_qkv"][2 * H + g * GD:2 * H + (g + 1) * GD].astype(np.float64)
    wout_g = inputs["w_out"][g * GD:(g + 1) * GD].astype(np.float64)
    out = y.astype(np.float64) + bv_g @ wout_g
    if g == 0:
        out = out + inputs["b_out"].astype(np.float64)
    return out.astype(np.float32)


def _mask_chunks(mask):
    """ceil(max valid length / 128) over the batch -- the NEFF specializer."""
    lens = np.asarray(mask).astype(np.int64).sum(axis=1)
    return int(max(1, min(KC, int(np.ceil(lens.max() / 128.0)))))


def kernel(input, mask, w_qkv, b_qkv, w_out, b_out):
    from concourse.bass_utils import run_bass_kernel_spmd

    input = np.asarray(input)
    mask = np.asarray(mask)
    w_qkv = np.asarray(w_qkv)
    b_qkv = np.asarray(b_qkv)
    w_out = np.asarray(w_out)
    b_out = np.asarray(b_out)
    nc = _get_nc(_mask_chunks(mask))
    in_maps = [
        make_core_inputs(input, mask, w_qkv, b_qkv, w_out, b_out, c)
        for c in range(NCORES)
    ]
    res = run_bass_kernel_spmd(nc, in_maps, list(range(NCORES)))
    bias = _host_bias(w_qkv, b_qkv, w_out, b_out)
    parts = [res.results[c]["y"] for c in range(NCORES)]
    out = np.stack([parts[2 * b] + parts[2 * b + 1] + bias for b in range(B)])
    return out.astype(np.float32)


if __name__ == "__main__":
    nc = _build_nc(13)
    print("build ok")
